# revision 9
# baseline (speedup 1.0000x reference)
"""Trainium2 Bass kernel for GridSmoother: per-batch SPD grid-Laplacian solve.

System: L = I + Dx^T Wx Dx + Dy^T Wy Dy over a 48x64 grid, solved for 16
channels per batch, B=4 batches.  lambda(L) in [1, 1+2*max_node(sum w)] --
tiny condition number, so a fixed-coefficient Chebyshev iteration on the
5-point stencil converges at ~0.5x error per iteration.

Sharding: batch b -> cores {2b, 2b+1}, each core owns 8 channels.
Per-core data layout (SBUF tile [128 partitions, 194 free]):
  partition p = (c_local//4)*64 + w      (c_hi in {0,1}, w in 0..63)
  free      f = 1 + (c_local%4)*48 + h   (c_lo in {0..3}, h in 0..47)
  f=0 and f=193 are zero guard columns.
Vertical (h+-1) neighbor access = free-dim offset reads (guards + zeroed
boundary weights make wraps harmless).  Horizontal (w+-1) = partition shifts
done on the TensorEngine with block-diagonal +-1 shift matrices, accumulated
in PSUM together with the diagonal and vertical terms (5 matmuls -> P = A*u).
"""

import numpy as np
import sys

sys.path.insert(0, "/opt/trn_rl_repo")

import concourse.bass as bass
from concourse import mybir
from concourse.bass_utils import run_bass_kernel_spmd

B, C, H, W = 4, 16, 48, 64
NCORE = 8
CPC = C // 2          # channels per core = 8
FD = 194              # free dim incl. 2 guards
FDA = 192             # active free size
NBLK = 5              # wxz, wxzUP, wyz, wyzUP, diag

F32 = mybir.dt.float32

_COMPILED = {}


def _planes(wx, wy):
    """Per-batch weight planes in (h, w) image space."""
    wxz = wx.copy()
    wxz[:, -1] = 0.0
    wyz = wy.copy()
    wyz[-1, :] = 0.0
    wxzUP = np.zeros_like(wxz)
    wxzUP[:, 1:] = wxz[:, :-1]
    wyzUP = np.zeros_like(wyz)
    wyzUP[1:, :] = wyz[:-1, :]
    diag = 1.0 + wxz + wxzUP + wyz + wyzUP
    return wxz, wxzUP, wyz, wyzUP, diag


def _plane2core(plane_hw):
    """[48,64] (h,w) plane -> [128,194] core layout with zero guards."""
    out = np.zeros((128, FD), dtype=np.float32)
    t = plane_hw.T  # [w, h] = [64, 48]
    out[:, 1:193] = np.tile(t, (2, 4))
    return out


def _b2core(ae_chans):
    """[8,48,64] -> [128,192]."""
    a = ae_chans.reshape(2, 4, H, W)
    a = np.transpose(a, (0, 3, 1, 2))  # [c_hi, w, c_lo, h]
    return np.ascontiguousarray(a.reshape(128, FDA), dtype=np.float32)


def _core2out(xt):
    """[128,192] -> [8,48,64]."""
    a = xt.reshape(2, W, 4, H)
    a = np.transpose(a, (0, 2, 3, 1))  # [c_hi, c_lo, h, w]
    return a.reshape(CPC, H, W)


def _shift_mats():
    """lhsT matrices [128,128]: I(+1), SupN(-1 at k=m-1), SdnN(-1 at k=m+1),
    IN(-I). Block-diagonal over the two 64-partition halves."""
    ipos = np.eye(128, dtype=np.float32)
    sup = np.zeros((128, 128), dtype=np.float32)
    sdn = np.zeros((128, 128), dtype=np.float32)
    for m in range(128):
        if m % 64 != 0:
            sup[m - 1, m] = -1.0
        if m % 64 != 63:
            sdn[m + 1, m] = -1.0
    ineg = -ipos
    return np.concatenate([ipos, sup, sdn, ineg], axis=1)  # [128, 512]


def _cheby_coeffs(lam_max, n_iter):
    """Returns per-iteration (gamma_k, c_next_k) for the scaled-direction
    Chebyshev recurrence:
        x += gamma_k * u ; r -= gamma_k * A u ; u = c_{k+1} * u + r
    """
    lmin = 1.0
    theta = (lam_max + lmin) / 2.0
    delta = (lam_max - lmin) / 2.0
    sigma1 = theta / delta
    gammas, cnexts = [], []
    gamma = 1.0 / theta
    rho = 1.0 / sigma1
    for _ in range(n_iter):
        rho_next = 1.0 / (2.0 * sigma1 - rho)
        c_next = rho * gamma * delta / 2.0
        gamma_next = 2.0 * rho_next / delta
        gammas.append(gamma)
        cnexts.append(c_next)
        rho, gamma = rho_next, gamma_next
    return gammas, cnexts


N_ITER = 24


def _build(lam_max, n_iter):
    """Raw Bass program (explicit semaphores; every instruction carries at
    most one wait -- the walrus codegen on this path rejects multi-wait
    sync_info)."""
    nc = bass.Bass("TRN2", target_bir_lowering=False, debug=False,
                   num_devices=NCORE, detect_race_conditions=False)
    bt_d = nc.dram_tensor("bt", [128, FDA], F32, kind="ExternalInput").ap()
    wcat_d = nc.dram_tensor("wcat", [128, NBLK * FD], F32,
                            kind="ExternalInput").ap()
    smats_d = nc.dram_tensor("smats", [128, 512], F32,
                             kind="ExternalInput").ap()
    xout_d = nc.dram_tensor("xout", [128, FDA], F32,
                            kind="ExternalOutput").ap()

    gammas, cnexts = _cheby_coeffs(lam_max, n_iter)
    theta = (lam_max + 1.0) / 2.0

    wcat = nc.alloc_sbuf_tensor("wcat_s", [128, NBLK * FD], F32).ap()
    smats = nc.alloc_sbuf_tensor("smats_s", [128, 512], F32).ap()
    btile = nc.alloc_sbuf_tensor("btile_s", [128, FDA], F32).ap()
    r = nc.alloc_sbuf_tensor("r_s", [128, FD], F32).ap()
    u = nc.alloc_sbuf_tensor("u_s", [128, FD], F32).ap()
    x = nc.alloc_sbuf_tensor("x_s", [128, FD], F32).ap()
    pc = nc.alloc_sbuf_tensor("pc_s", [128, NBLK * FD], F32).ap()
    P = nc.alloc_psum_tensor("P_s", [128, FDA], F32).ap()

    mI = smats[:, 0:128]
    mSup = smats[:, 128:256]
    mSdn = smats[:, 256:384]
    mIN = smats[:, 384:512]

    u_b = u.rearrange("p (o f) -> p o f", o=1).broadcast_to([128, NBLK, FD])
    w_b = wcat.rearrange("p (o f) -> p o f", o=NBLK)
    pc_b = pc.rearrange("p (o f) -> p o f", o=NBLK)

    dma_sem = nc.alloc_semaphore("dma_sem")
    dve_sem = nc.alloc_semaphore("dve_sem")   # counts pc-ready TTs
    pe_sem = nc.alloc_semaphore("pe_sem")     # counts matmuls
    gp_sem = nc.alloc_semaphore("gp_sem")     # x memset done
    out_sem = nc.alloc_semaphore("out_sem")   # final x ready

    with nc.Block() as block:

        @block.gpsimd
        def _(gp):
            gp.dma_start(wcat, wcat_d).then_inc(dma_sem, 16)
            gp.dma_start(smats, smats_d).then_inc(dma_sem, 16)
            gp.dma_start(btile, bt_d).then_inc(dma_sem, 16)
            gp.memset(x, 0.0).then_inc(gp_sem, 1)
            gp.wait_ge(out_sem, 1)
            gp.dma_start(xout_d, x[:, 1:193]).then_inc(dma_sem, 16)
            gp.wait_ge(dma_sem, 64)

        @block.tensor
        def _(pe):
            pe.wait_ge(dma_sem, 48)  # all inputs loaded
            for k in range(n_iter - 1):
                pe.wait_ge(dve_sem, k + 1)
                pe.matmul(P, mI, pc[:, 4 * FD + 1:4 * FD + 193],
                          start=True, stop=False).then_inc(pe_sem, 1)
                pe.matmul(P, mSup, pc[:, 0 * FD + 1:0 * FD + 193],
                          start=False, stop=False).then_inc(pe_sem, 1)
                pe.matmul(P, mSdn, pc[:, 1 * FD + 1:1 * FD + 193],
                          start=False, stop=False).then_inc(pe_sem, 1)
                pe.matmul(P, mIN, pc[:, 2 * FD + 0:2 * FD + 192],
                          start=False, stop=False).then_inc(pe_sem, 1)
                pe.matmul(P, mIN, pc[:, 3 * FD + 2:3 * FD + 194],
                          start=False, stop=True).then_inc(pe_sem, 1)

        @block.vector
        def _(v):
            v.wait_ge(dma_sem, 48)  # btile (and everything before) loaded
            v.memset(r, 0.0)
            v.tensor_copy(r[:, 1:193], btile)
            v.tensor_scalar_mul(u, r, 1.0 / theta)
            v.wait_ge(gp_sem, 1)    # x memset done
            for k in range(n_iter):
                g = float(gammas[k])
                if k == n_iter - 1:
                    v.scalar_tensor_tensor(
                        x, u, g, x,
                        mybir.AluOpType.mult,
                        mybir.AluOpType.add).then_inc(out_sem, 1)
                    break
                c = float(cnexts[k])
                v.tensor_tensor(pc_b, w_b, u_b,
                                mybir.AluOpType.mult).then_inc(dve_sem, 1)
                # x += gamma * u (runs while PE computes A u)
                v.scalar_tensor_tensor(x, u, g, x,
                                       mybir.AluOpType.mult,
                                       mybir.AluOpType.add)
                v.wait_ge(pe_sem, 5 * (k + 1))
                # r -= gamma * P
                v.scalar_tensor_tensor(r[:, 1:193], P, -g, r[:, 1:193],
                                       mybir.AluOpType.mult,
                                       mybir.AluOpType.add)
                # u = c_next * u + r
                v.scalar_tensor_tensor(u, u, c, r,
                                       mybir.AluOpType.mult,
                                       mybir.AluOpType.add)

    return nc


def kernel(ae: np.ndarray, wxwy: np.ndarray) -> np.ndarray:
    ae = np.asarray(ae, dtype=np.float32)
    wxwy = np.asarray(wxwy, dtype=np.float32)

    # ---- host prep: per-core shards -------------------------------------
    smats = _shift_mats()
    in_maps = []
    lam_max = 0.0
    wcats = []
    for b in range(B):
        wxz, wxzUP, wyz, wyzUP, diag = _planes(wxwy[b, 0], wxwy[b, 1])
        inc = wxz + wxzUP + wyz + wyzUP
        lam_max = max(lam_max, 1.0 + 2.0 * float(inc.max()))
        wcats.append(np.concatenate(
            [_plane2core(p) for p in (wxz, wxzUP, wyz, wyzUP, diag)], axis=1))
    # round lam_max up a touch for a safe, cache-friendly constant
    lam_max = float(np.ceil(lam_max * 64.0) / 64.0)

    for core in range(NCORE):
        b, half = core // 2, core % 2
        bt = _b2core(ae[b, half * CPC:(half + 1) * CPC])
        in_maps.append({"bt": bt, "wcat": wcats[b], "smats": smats})

    key = (lam_max, N_ITER)
    if key not in _COMPILED:
        _COMPILED[key] = _build(lam_max, N_ITER)
    nc = _COMPILED[key]

    global _LAST_BUILD
    _LAST_BUILD = (nc, in_maps)

    res = run_bass_kernel_spmd(nc, in_maps, list(range(NCORE)))

    out = np.empty((B, C, H, W), dtype=np.float32)
    for core in range(NCORE):
        b, half = core // 2, core % 2
        out[b, half * CPC:(half + 1) * CPC] = _core2out(
            res.results[core]["xout"])
    return out


# revision 10
# speedup vs baseline: 1.0507x; 1.0507x over previous
"""Trainium2 Bass kernel for GridSmoother: per-batch SPD grid-Laplacian solve.

System: L = I + Dx^T Wx Dx + Dy^T Wy Dy over a 48x64 grid, solved for 16
channels per batch, B=4 batches.  lambda(L) in [1, 1+2*max_node(sum w)] --
tiny condition number, so a fixed-coefficient Chebyshev iteration on the
5-point stencil converges at ~0.5x error per iteration.

Sharding: batch b -> cores {2b, 2b+1}, each core owns 8 channels.
Per-core data layout (SBUF tile [128 partitions, 194 free]):
  partition p = (c_local//4)*64 + w      (c_hi in {0,1}, w in 0..63)
  free      f = 1 + (c_local%4)*48 + h   (c_lo in {0..3}, h in 0..47)
  f=0 and f=193 are zero guard columns.
Vertical (h+-1) neighbor access = free-dim offset reads (guards + zeroed
boundary weights make wraps harmless).  Horizontal (w+-1) = partition shifts
done on the TensorEngine with block-diagonal +-1 shift matrices, accumulated
in PSUM together with the diagonal and vertical terms (5 matmuls -> P = A*u).
"""

import numpy as np
import sys

sys.path.insert(0, "/opt/trn_rl_repo")

import concourse.bass as bass
from concourse import mybir
from concourse.bass_utils import run_bass_kernel_spmd

B, C, H, W = 4, 16, 48, 64
NCORE = 8
CPC = C // 2          # channels per core = 8
FD = 194              # free dim incl. 2 guards
FDA = 192             # active free size
NBLK = 5              # wxz, wxzUP, wyz, wyzUP, diag

F32 = mybir.dt.float32

_COMPILED = {}


def _planes(wx, wy):
    """Per-batch weight planes in (h, w) image space."""
    wxz = wx.copy()
    wxz[:, -1] = 0.0
    wyz = wy.copy()
    wyz[-1, :] = 0.0
    wxzUP = np.zeros_like(wxz)
    wxzUP[:, 1:] = wxz[:, :-1]
    wyzUP = np.zeros_like(wyz)
    wyzUP[1:, :] = wyz[:-1, :]
    diag = 1.0 + wxz + wxzUP + wyz + wyzUP
    return wxz, wxzUP, wyz, wyzUP, diag


def _plane2core(plane_hw):
    """[48,64] (h,w) plane -> [128,194] core layout with zero guards."""
    out = np.zeros((128, FD), dtype=np.float32)
    t = plane_hw.T  # [w, h] = [64, 48]
    out[:, 1:193] = np.tile(t, (2, 4))
    return out


def _b2core(ae_chans):
    """[8,48,64] -> [128,192]."""
    a = ae_chans.reshape(2, 4, H, W)
    a = np.transpose(a, (0, 3, 1, 2))  # [c_hi, w, c_lo, h]
    return np.ascontiguousarray(a.reshape(128, FDA), dtype=np.float32)


def _core2out(xt):
    """[128,192] -> [8,48,64]."""
    a = xt.reshape(2, W, 4, H)
    a = np.transpose(a, (0, 2, 3, 1))  # [c_hi, c_lo, h, w]
    return a.reshape(CPC, H, W)


def _shift_mats():
    """lhsT matrices [128,128]: I(+1), SupN(-1 at k=m-1), SdnN(-1 at k=m+1),
    IN(-I). Block-diagonal over the two 64-partition halves."""
    ipos = np.eye(128, dtype=np.float32)
    sup = np.zeros((128, 128), dtype=np.float32)
    sdn = np.zeros((128, 128), dtype=np.float32)
    for m in range(128):
        if m % 64 != 0:
            sup[m - 1, m] = -1.0
        if m % 64 != 63:
            sdn[m + 1, m] = -1.0
    ineg = -ipos
    return np.concatenate([ipos, sup, sdn, ineg], axis=1)  # [128, 512]


def _cheby_coeffs(lam_max, n_iter):
    """Returns per-iteration (gamma_k, c_next_k) for the scaled-direction
    Chebyshev recurrence:
        x += gamma_k * u ; r -= gamma_k * A u ; u = c_{k+1} * u + r
    """
    lmin = 1.0
    theta = (lam_max + lmin) / 2.0
    delta = (lam_max - lmin) / 2.0
    sigma1 = theta / delta
    gammas, cnexts = [], []
    gamma = 1.0 / theta
    rho = 1.0 / sigma1
    for _ in range(n_iter):
        rho_next = 1.0 / (2.0 * sigma1 - rho)
        c_next = rho * gamma * delta / 2.0
        gamma_next = 2.0 * rho_next / delta
        gammas.append(gamma)
        cnexts.append(c_next)
        rho, gamma = rho_next, gamma_next
    return gammas, cnexts


N_ITER = 20


def _build(lam_max, n_iter):
    """Raw Bass program (explicit semaphores; every instruction carries at
    most one wait -- the walrus codegen on this path rejects multi-wait
    sync_info)."""
    nc = bass.Bass("TRN2", target_bir_lowering=False, debug=False,
                   num_devices=NCORE, detect_race_conditions=False)
    bt_d = nc.dram_tensor("bt", [128, FDA], F32, kind="ExternalInput").ap()
    wcat_d = nc.dram_tensor("wcat", [128, NBLK * FD], F32,
                            kind="ExternalInput").ap()
    smats_d = nc.dram_tensor("smats", [128, 512], F32,
                             kind="ExternalInput").ap()
    xout_d = nc.dram_tensor("xout", [128, FDA], F32,
                            kind="ExternalOutput").ap()

    gammas, cnexts = _cheby_coeffs(lam_max, n_iter)
    theta = (lam_max + 1.0) / 2.0

    wcat = nc.alloc_sbuf_tensor("wcat_s", [128, NBLK * FD], F32).ap()
    smats = nc.alloc_sbuf_tensor("smats_s", [128, 512], F32).ap()
    btile = nc.alloc_sbuf_tensor("btile_s", [128, FDA], F32).ap()
    r = nc.alloc_sbuf_tensor("r_s", [128, FD], F32).ap()
    u = nc.alloc_sbuf_tensor("u_s", [128, FD], F32).ap()
    x = nc.alloc_sbuf_tensor("x_s", [128, FD], F32).ap()
    pc = nc.alloc_sbuf_tensor("pc_s", [128, NBLK * FD], F32).ap()
    P = nc.alloc_psum_tensor("P_s", [128, FDA], F32).ap()

    mI = smats[:, 0:128]
    mSup = smats[:, 128:256]
    mSdn = smats[:, 256:384]
    mIN = smats[:, 384:512]

    u_b = u.rearrange("p (o f) -> p o f", o=1).broadcast_to([128, NBLK, FD])
    w_b = wcat.rearrange("p (o f) -> p o f", o=NBLK)
    pc_b = pc.rearrange("p (o f) -> p o f", o=NBLK)

    dma_sem = nc.alloc_semaphore("dma_sem")
    dve_sem = nc.alloc_semaphore("dve_sem")   # counts pc-ready TTs
    pe_sem = nc.alloc_semaphore("pe_sem")     # counts matmuls
    gp_sem = nc.alloc_semaphore("gp_sem")     # x memset done
    out_sem = nc.alloc_semaphore("out_sem")   # final x ready

    with nc.Block() as block:

        @block.gpsimd
        def _(gp):
            gp.dma_start(wcat, wcat_d).then_inc(dma_sem, 16)
            gp.dma_start(smats, smats_d).then_inc(dma_sem, 16)
            gp.dma_start(btile, bt_d).then_inc(dma_sem, 16)
            gp.memset(x, 0.0).then_inc(gp_sem, 1)
            gp.wait_ge(out_sem, 1)
            gp.dma_start(xout_d, x[:, 1:193]).then_inc(dma_sem, 16)
            gp.wait_ge(dma_sem, 64)

        @block.tensor
        def _(pe):
            pe.wait_ge(dma_sem, 48)  # all inputs loaded
            for k in range(n_iter - 1):
                pe.wait_ge(dve_sem, k + 1)
                pe.matmul(P, mI, pc[:, 4 * FD + 1:4 * FD + 193],
                          start=True, stop=False).then_inc(pe_sem, 1)
                pe.matmul(P, mSup, pc[:, 0 * FD + 1:0 * FD + 193],
                          start=False, stop=False).then_inc(pe_sem, 1)
                pe.matmul(P, mSdn, pc[:, 1 * FD + 1:1 * FD + 193],
                          start=False, stop=False).then_inc(pe_sem, 1)
                pe.matmul(P, mIN, pc[:, 2 * FD + 0:2 * FD + 192],
                          start=False, stop=False).then_inc(pe_sem, 1)
                pe.matmul(P, mIN, pc[:, 3 * FD + 2:3 * FD + 194],
                          start=False, stop=True).then_inc(pe_sem, 1)

        @block.vector
        def _(v):
            v.wait_ge(dma_sem, 48)  # btile (and everything before) loaded
            v.memset(r, 0.0)
            v.tensor_copy(r[:, 1:193], btile)
            v.tensor_scalar_mul(u, r, 1.0 / theta)
            v.wait_ge(gp_sem, 1)    # x memset done
            for k in range(n_iter):
                g = float(gammas[k])
                if k == n_iter - 1:
                    v.scalar_tensor_tensor(
                        x, u, g, x,
                        mybir.AluOpType.mult,
                        mybir.AluOpType.add).then_inc(out_sem, 1)
                    break
                c = float(cnexts[k])
                v.tensor_tensor(pc_b, w_b, u_b,
                                mybir.AluOpType.mult).then_inc(dve_sem, 1)
                # x += gamma * u (runs while PE computes A u)
                v.scalar_tensor_tensor(x, u, g, x,
                                       mybir.AluOpType.mult,
                                       mybir.AluOpType.add)
                v.wait_ge(pe_sem, 5 * (k + 1))
                # r -= gamma * P
                v.scalar_tensor_tensor(r[:, 1:193], P, -g, r[:, 1:193],
                                       mybir.AluOpType.mult,
                                       mybir.AluOpType.add)
                # u = c_next * u + r
                v.scalar_tensor_tensor(u, u, c, r,
                                       mybir.AluOpType.mult,
                                       mybir.AluOpType.add)

    return nc


def kernel(ae: np.ndarray, wxwy: np.ndarray) -> np.ndarray:
    ae = np.asarray(ae, dtype=np.float32)
    wxwy = np.asarray(wxwy, dtype=np.float32)

    # ---- host prep: per-core shards -------------------------------------
    smats = _shift_mats()
    in_maps = []
    lam_max = 0.0
    wcats = []
    for b in range(B):
        wxz, wxzUP, wyz, wyzUP, diag = _planes(wxwy[b, 0], wxwy[b, 1])
        inc = wxz + wxzUP + wyz + wyzUP
        lam_max = max(lam_max, 1.0 + 2.0 * float(inc.max()))
        wcats.append(np.concatenate(
            [_plane2core(p) for p in (wxz, wxzUP, wyz, wyzUP, diag)], axis=1))
    # round lam_max up a touch for a safe, cache-friendly constant
    lam_max = float(np.ceil(lam_max * 64.0) / 64.0)

    for core in range(NCORE):
        b, half = core // 2, core % 2
        bt = _b2core(ae[b, half * CPC:(half + 1) * CPC])
        in_maps.append({"bt": bt, "wcat": wcats[b], "smats": smats})

    key = (lam_max, N_ITER)
    if key not in _COMPILED:
        _COMPILED[key] = _build(lam_max, N_ITER)
    nc = _COMPILED[key]

    global _LAST_BUILD
    _LAST_BUILD = (nc, in_maps)

    res = run_bass_kernel_spmd(nc, in_maps, list(range(NCORE)))

    out = np.empty((B, C, H, W), dtype=np.float32)
    for core in range(NCORE):
        b, half = core // 2, core % 2
        out[b, half * CPC:(half + 1) * CPC] = _core2out(
            res.results[core]["xout"])
    return out


# revision 11
# speedup vs baseline: 1.2337x; 1.1741x over previous
"""Trainium2 Bass kernel for GridSmoother: per-batch SPD grid-Laplacian solve.

System: L = I + Dx^T Wx Dx + Dy^T Wy Dy over a 48x64 grid, solved for 16
channels per batch, B=4 batches.  lambda(L) in [1, 1+2*max_node(sum w)] --
tiny condition number, so a fixed-coefficient Chebyshev iteration on the
5-point stencil converges at ~0.5x error per iteration.

Sharding: batch b -> cores {2b, 2b+1}, each core owns 8 channels.
Per-core data layout (SBUF tile [128 partitions, 194 free]):
  partition p = (c_local//4)*64 + w      (c_hi in {0,1}, w in 0..63)
  free      f = 1 + (c_local%4)*48 + h   (c_lo in {0..3}, h in 0..47)
  f=0 and f=193 are zero guard columns.
Vertical (h+-1) neighbor access = free-dim offset reads (guards + zeroed
boundary weights make wraps harmless).  Horizontal (w+-1) = partition shifts
done on the TensorEngine with block-diagonal +-1 shift matrices, accumulated
in PSUM together with the diagonal and vertical terms (5 matmuls -> P = A*u).
"""

import numpy as np
import sys

sys.path.insert(0, "/opt/trn_rl_repo")

import concourse.bass as bass
from concourse import mybir
from concourse.bass_utils import run_bass_kernel_spmd

B, C, H, W = 4, 16, 48, 64
NCORE = 8
CPC = C // 2          # channels per core = 8
FD = 194              # free dim incl. 2 guards
FDA = 192             # active free size
NBLK = 5              # wxz, wxzUP, wyz, wyzUP, diag

F32 = mybir.dt.float32

_COMPILED = {}


def _planes(wx, wy):
    """Per-batch weight planes in (h, w) image space."""
    wxz = wx.copy()
    wxz[:, -1] = 0.0
    wyz = wy.copy()
    wyz[-1, :] = 0.0
    wxzUP = np.zeros_like(wxz)
    wxzUP[:, 1:] = wxz[:, :-1]
    wyzUP = np.zeros_like(wyz)
    wyzUP[1:, :] = wyz[:-1, :]
    diag = 1.0 + wxz + wxzUP + wyz + wyzUP
    return wxz, wxzUP, wyz, wyzUP, diag


def _plane2core(plane_hw):
    """[48,64] (h,w) plane -> [128,194] core layout with zero guards."""
    out = np.zeros((128, FD), dtype=np.float32)
    t = plane_hw.T  # [w, h] = [64, 48]
    out[:, 1:193] = np.tile(t, (2, 4))
    return out


def _b2core(ae_chans):
    """[8,48,64] -> [128,192]."""
    a = ae_chans.reshape(2, 4, H, W)
    a = np.transpose(a, (0, 3, 1, 2))  # [c_hi, w, c_lo, h]
    return np.ascontiguousarray(a.reshape(128, FDA), dtype=np.float32)


def _core2out(xt):
    """[128,192] -> [8,48,64]."""
    a = xt.reshape(2, W, 4, H)
    a = np.transpose(a, (0, 2, 3, 1))  # [c_hi, c_lo, h, w]
    return a.reshape(CPC, H, W)


def _shift_mats():
    """lhsT matrices [128,128]: I(+1), SupN(-1 at k=m-1), SdnN(-1 at k=m+1),
    IN(-I). Block-diagonal over the two 64-partition halves."""
    ipos = np.eye(128, dtype=np.float32)
    sup = np.zeros((128, 128), dtype=np.float32)
    sdn = np.zeros((128, 128), dtype=np.float32)
    for m in range(128):
        if m % 64 != 0:
            sup[m - 1, m] = -1.0
        if m % 64 != 63:
            sdn[m + 1, m] = -1.0
    ineg = -ipos
    return np.concatenate([ipos, sup, sdn, ineg], axis=1)  # [128, 512]


def _cheby_coeffs(lam_max, n_iter):
    """Returns per-iteration (gamma_k, c_next_k) for the scaled-direction
    Chebyshev recurrence:
        x += gamma_k * u ; r -= gamma_k * A u ; u = c_{k+1} * u + r
    """
    lmin = 1.0
    theta = (lam_max + lmin) / 2.0
    delta = (lam_max - lmin) / 2.0
    sigma1 = theta / delta
    gammas, cnexts = [], []
    gamma = 1.0 / theta
    rho = 1.0 / sigma1
    for _ in range(n_iter):
        rho_next = 1.0 / (2.0 * sigma1 - rho)
        c_next = rho * gamma * delta / 2.0
        gamma_next = 2.0 * rho_next / delta
        gammas.append(gamma)
        cnexts.append(c_next)
        rho, gamma = rho_next, gamma_next
    return gammas, cnexts


N_ITER = 20


def _build(lam_max, n_iter):
    """Raw Bass program (explicit semaphores; every instruction carries at
    most one wait -- the walrus codegen on this path rejects multi-wait
    sync_info)."""
    nc = bass.Bass("TRN2", target_bir_lowering=False, debug=False,
                   num_devices=NCORE, detect_race_conditions=False)
    bt_d = nc.dram_tensor("bt", [128, FDA], F32, kind="ExternalInput").ap()
    wcat_d = nc.dram_tensor("wcat", [128, NBLK * FD], F32,
                            kind="ExternalInput").ap()
    smats_d = nc.dram_tensor("smats", [128, 512], F32,
                             kind="ExternalInput").ap()
    xout_d = nc.dram_tensor("xout", [128, FDA], F32,
                            kind="ExternalOutput").ap()

    gammas, cnexts = _cheby_coeffs(lam_max, n_iter)
    theta = (lam_max + 1.0) / 2.0

    wcat = nc.alloc_sbuf_tensor("wcat_s", [128, NBLK * FD], F32).ap()
    smats = nc.alloc_sbuf_tensor("smats_s", [128, 512], F32).ap()
    btile = nc.alloc_sbuf_tensor("btile_s", [128, FDA], F32).ap()
    r = nc.alloc_sbuf_tensor("r_s", [128, FD], F32).ap()
    u = nc.alloc_sbuf_tensor("u_s", [128, FD], F32).ap()
    x = nc.alloc_sbuf_tensor("x_s", [128, FD], F32).ap()
    pc = nc.alloc_sbuf_tensor("pc_s", [128, NBLK * FD], F32).ap()
    P = nc.alloc_psum_tensor("P_s", [128, FDA], F32).ap()

    mI = smats[:, 0:128]
    mSup = smats[:, 128:256]
    mSdn = smats[:, 256:384]
    mIN = smats[:, 384:512]

    u_b = u.rearrange("p (o f) -> p o f", o=1).broadcast_to([128, NBLK, FD])
    w_b = wcat.rearrange("p (o f) -> p o f", o=NBLK)
    pc_b = pc.rearrange("p (o f) -> p o f", o=NBLK)

    dma_sem = nc.alloc_semaphore("dma_sem")
    dve_sem = nc.alloc_semaphore("dve_sem")   # counts pc-ready TTs
    pe_sem = nc.alloc_semaphore("pe_sem")     # counts matmuls
    gp_sem = nc.alloc_semaphore("gp_sem")     # x memset done
    out_sem = nc.alloc_semaphore("out_sem")   # final x ready

    with nc.Block() as block:

        @block.gpsimd
        def _(gp):
            gp.dma_start(wcat, wcat_d).then_inc(dma_sem, 16)
            gp.dma_start(smats, smats_d).then_inc(dma_sem, 16)
            gp.dma_start(btile, bt_d).then_inc(dma_sem, 16)
            gp.memset(x, 0.0).then_inc(gp_sem, 1)
            gp.wait_ge(out_sem, 1)
            gp.dma_start(xout_d, x[:, 1:193]).then_inc(dma_sem, 16)
            gp.wait_ge(dma_sem, 64)

        @block.tensor
        def _(pe):
            pe.wait_ge(dma_sem, 48)  # all inputs loaded
            for k in range(n_iter - 1):
                pe.wait_ge(dve_sem, 2 * k + 1)
                pe.matmul(P, mSup, pc[:, 0 * FD + 1:0 * FD + 193],
                          start=True, stop=False).then_inc(pe_sem, 1)
                pe.matmul(P, mSdn, pc[:, 1 * FD + 1:1 * FD + 193],
                          start=False, stop=False).then_inc(pe_sem, 1)
                pe.wait_ge(dve_sem, 2 * k + 2)
                pe.matmul(P, mI, pc[:, 4 * FD + 1:4 * FD + 193],
                          start=False, stop=False).then_inc(pe_sem, 1)
                pe.matmul(P, mIN, pc[:, 2 * FD + 0:2 * FD + 192],
                          start=False, stop=False).then_inc(pe_sem, 1)
                pe.matmul(P, mIN, pc[:, 3 * FD + 2:3 * FD + 194],
                          start=False, stop=True).then_inc(pe_sem, 1)

        @block.vector
        def _(v):
            v.wait_ge(dma_sem, 48)  # btile (and everything before) loaded
            v.memset(r, 0.0)
            v.tensor_copy(r[:, 1:193], btile)
            v.tensor_scalar_mul(u, r, 1.0 / theta)
            v.wait_ge(gp_sem, 1)    # x memset done
            for k in range(n_iter):
                g = float(gammas[k])
                if k == n_iter - 1:
                    v.scalar_tensor_tensor(
                        x, u, g, x,
                        mybir.AluOpType.mult,
                        mybir.AluOpType.add).then_inc(out_sem, 1)
                    break
                c = float(cnexts[k])
                u_b2 = u.rearrange("p (o f) -> p o f", o=1).broadcast_to(
                    [128, 2, FD])
                u_b3 = u.rearrange("p (o f) -> p o f", o=1).broadcast_to(
                    [128, 3, FD])
                v.tensor_tensor(
                    pc[:, 0:2 * FD].rearrange("p (o f) -> p o f", o=2),
                    wcat[:, 0:2 * FD].rearrange("p (o f) -> p o f", o=2),
                    u_b2, mybir.AluOpType.mult).then_inc(dve_sem, 1)
                v.tensor_tensor(
                    pc[:, 2 * FD:5 * FD].rearrange("p (o f) -> p o f", o=3),
                    wcat[:, 2 * FD:5 * FD].rearrange("p (o f) -> p o f", o=3),
                    u_b3, mybir.AluOpType.mult).then_inc(dve_sem, 1)
                # x += gamma * u (runs while PE computes A u)
                v.scalar_tensor_tensor(x, u, g, x,
                                       mybir.AluOpType.mult,
                                       mybir.AluOpType.add)
                v.wait_ge(pe_sem, 5 * (k + 1))
                # r -= gamma * P
                v.scalar_tensor_tensor(r[:, 1:193], P, -g, r[:, 1:193],
                                       mybir.AluOpType.mult,
                                       mybir.AluOpType.add)
                # u = c_next * u + r
                v.scalar_tensor_tensor(u, u, c, r,
                                       mybir.AluOpType.mult,
                                       mybir.AluOpType.add)

    return nc


def kernel(ae: np.ndarray, wxwy: np.ndarray) -> np.ndarray:
    ae = np.asarray(ae, dtype=np.float32)
    wxwy = np.asarray(wxwy, dtype=np.float32)

    # ---- host prep: per-core shards -------------------------------------
    smats = _shift_mats()
    in_maps = []
    lam_max = 0.0
    wcats = []
    for b in range(B):
        wxz, wxzUP, wyz, wyzUP, diag = _planes(wxwy[b, 0], wxwy[b, 1])
        inc = wxz + wxzUP + wyz + wyzUP
        lam_max = max(lam_max, 1.0 + 2.0 * float(inc.max()))
        wcats.append(np.concatenate(
            [_plane2core(p) for p in (wxz, wxzUP, wyz, wyzUP, diag)], axis=1))
    # round lam_max up a touch for a safe, cache-friendly constant
    lam_max = float(np.ceil(lam_max * 64.0) / 64.0)

    for core in range(NCORE):
        b, half = core // 2, core % 2
        bt = _b2core(ae[b, half * CPC:(half + 1) * CPC])
        in_maps.append({"bt": bt, "wcat": wcats[b], "smats": smats})

    key = (lam_max, N_ITER)
    if key not in _COMPILED:
        _COMPILED[key] = _build(lam_max, N_ITER)
    nc = _COMPILED[key]

    global _LAST_BUILD
    _LAST_BUILD = (nc, in_maps)

    res = run_bass_kernel_spmd(nc, in_maps, list(range(NCORE)))

    out = np.empty((B, C, H, W), dtype=np.float32)
    for core in range(NCORE):
        b, half = core // 2, core % 2
        out[b, half * CPC:(half + 1) * CPC] = _core2out(
            res.results[core]["xout"])
    return out
